# revision 11
# baseline (speedup 1.0000x reference)
"""Trainium2 Bass kernel for nn_AttentionModule (gnn_message_passing).

Sharding: 8 cores = 4 batches x 2 N-halves. Each core handles one batch's
N-half (2048 points). GroupNorm statistics are global over (C/G, N, K), so
each core computes partial per-channel sums and a 2-core AllReduce (within
the batch pair) produces exact global stats.

Device algorithm (3 streaming phases, GN affines folded into 1x1 convs):
  A: stream grouped_feat -> u = relu(concat(W_feat@feat bcast_K, W_grp@gf) + b)
     kept resident in SBUF (f16), bn_stats(u) -> AllReduce -> GN1 stats.
  B: z2 = (GN1-folded W1)@u + b1eff, v = relu(z2) spilled to HBM (f16),
     bn_stats(v); stream grouped_feat_out -> z3 = W_fo@gfo + b_fo kept
     resident (reusing u's SBUF slots), bn_stats(z3) -> AllReduce.
  C: scores = (GN2-folded W2)@v + mask(-60000 rank-1 matmul) + b2eff,
     p = exp(scores); gact = relu(s3*z3 + t3)  [GN3 affine];
     out = sum_k(p*gact) / sum_k(p) via f16 strided tree reduces.
"""
import numpy as np
import concourse.bacc as bacc
import concourse.bass as bass
import concourse.mybir as mybir
import concourse.tile as tile
from concourse.bass_utils import run_bass_kernel_spmd

dt = mybir.dt
AF = mybir.ActivationFunctionType
ALU = mybir.AluOpType

# Problem constants (hardcoded per spec)
B, C, N, K = 4, 128, 4096, 32
G, CPG = 32, 4
C1 = 64                      # W_feat / W_grp output channels
NLOC = N // 2                # 2048 points per core
FLOC = NLOC * K              # 65536 free positions per core
NPC = 64                     # points per chunk
CHUNK = NPC * K              # 2048 free positions per chunk
NCH = FLOC // CHUNK          # 32 chunks
EPS = 1e-5
CNT_TOTAL = float(CPG * N * K)   # global element count per (b, group)
MASKNEG = -60000.0

_CACHE = {}


def _stats_round(nc, tc, pools, tot_sb, ncol, gi_sb, git_sb):
    """From per-channel global sums tot_sb [C, ncol] (pairs of (sum, sumsq)
    columns), compute per-channel (inv, mu) [C,1] f32 for each pair.
    Returns list of (inv_c, mu_c) tiles."""
    sbuf, psum = pools
    res = []
    gp = psum.tile([G, ncol], dt.float32, tag="stp")
    nc.tensor.matmul(gp[:], gi_sb[:], tot_sb[:], start=True, stop=True)
    gsb = sbuf.tile([G, ncol], dt.float32, tag="sts")
    nc.vector.tensor_copy(gsb[:], gp[:])
    for j in range(ncol // 2):
        gmu = sbuf.tile([G, 1], dt.float32, tag="stm")
        nc.vector.tensor_scalar(gmu[:], gsb[:, 2 * j:2 * j + 1], 1.0 / CNT_TOTAL, None, ALU.mult)
        gmsq = sbuf.tile([G, 1], dt.float32, tag="stq")
        nc.vector.tensor_scalar(gmsq[:], gsb[:, 2 * j + 1:2 * j + 2], 1.0 / CNT_TOTAL, None, ALU.mult)
        gvar = sbuf.tile([G, 1], dt.float32, tag="stv")
        nc.vector.tensor_tensor(gvar[:], gmu[:], gmu[:], ALU.mult)
        nc.vector.tensor_tensor(gvar[:], gmsq[:], gvar[:], ALU.subtract)
        nc.vector.tensor_scalar_add(gvar[:], gvar[:], EPS)
        gstd = sbuf.tile([G, 1], dt.float32, tag="stsd")
        nc.scalar.activation(gstd[:], gvar[:], AF.Sqrt)
        ginv = sbuf.tile([G, 1], dt.float32, tag="stgi")
        nc.vector.reciprocal(ginv[:], gstd[:])
        # expand group -> channel via GIT matmul
        invp = psum.tile([C, 1], dt.float32, tag="stp2")
        nc.tensor.matmul(invp[:], git_sb[:], ginv[:], start=True, stop=True)
        inv_c = sbuf.tile([C, 1], dt.float32, tag="stic")
        nc.vector.tensor_copy(inv_c[:], invp[:])
        mup = psum.tile([C, 1], dt.float32, tag="stp3")
        nc.tensor.matmul(mup[:], git_sb[:], gmu[:], start=True, stop=True)
        mu_c = sbuf.tile([C, 1], dt.float32, tag="stmc")
        nc.vector.tensor_copy(mu_c[:], mup[:])
        res.append((inv_c, mu_c))
    return res


def _allreduce(nc, dram, sb_tile, ncol, groups):
    """AllReduce-add sb_tile [C, ncol] f32 across the replica pair; returns
    SBUF tile with the summed result."""
    bin_ = dram.tile([C, ncol], dt.float32, tag=f"arin{ncol}")
    bout = dram.tile([C, ncol], dt.float32, tag=f"arout{ncol}")
    nc.sync.dma_start(bin_[:], sb_tile[:])
    nc.gpsimd.collective_compute(
        "AllReduce", ALU.add, replica_groups=groups,
        ins=[bin_.opt()], outs=[bout.opt()],
    )
    tot = None
    return bout


def _build(n_cores):
    if n_cores in _CACHE:
        return _CACHE[n_cores]
    assert n_cores % 2 == 0
    groups = [[2 * i, 2 * i + 1] for i in range(n_cores // 2)]

    nc = bacc.Bacc("TRN2", target_bir_lowering=False, debug=False,
                   num_devices=n_cores)

    gf_d = nc.dram_tensor("gf", [C, NLOC, K], dt.float32, kind="ExternalInput")
    gfo_d = nc.dram_tensor("gfo", [C, NLOC, K], dt.float32, kind="ExternalInput")
    feat_d = nc.dram_tensor("feat", [C, NLOC], dt.float32, kind="ExternalInput")
    bigneg_d = nc.dram_tensor("bigneg", [NCH, CHUNK], dt.float16, kind="ExternalInput")
    wfeatT_d = nc.dram_tensor("wfeatT", [C, C1], dt.float32, kind="ExternalInput")
    wgrpT_d = nc.dram_tensor("wgrpT", [C, C1], dt.float32, kind="ExternalInput")
    w1T_d = nc.dram_tensor("w1T", [C, C], dt.float32, kind="ExternalInput")
    w2T_d = nc.dram_tensor("w2T", [C, C], dt.float32, kind="ExternalInput")
    wfoT_d = nc.dram_tensor("wfoT", [C, C], dt.float32, kind="ExternalInput")
    bcat_d = nc.dram_tensor("bcat", [C, 1], dt.float32, kind="ExternalInput")
    b1_d = nc.dram_tensor("b1", [C, 1], dt.float32, kind="ExternalInput")
    b2_d = nc.dram_tensor("b2", [C, 1], dt.float32, kind="ExternalInput")
    bfo_d = nc.dram_tensor("bfo", [C, 1], dt.float32, kind="ExternalInput")
    gn_d = {}
    for nm in ("gn1w", "gn1b", "gn2w", "gn2b", "gn3w", "gn3b"):
        gn_d[nm] = nc.dram_tensor(nm, [C, 1], dt.float32, kind="ExternalInput")
    gi_d = nc.dram_tensor("gi", [C, G], dt.float32, kind="ExternalInput")
    git_d = nc.dram_tensor("git", [G, C], dt.float32, kind="ExternalInput")
    onesc_d = nc.dram_tensor("onesc", [1, C], dt.float16, kind="ExternalInput")
    out_d = nc.dram_tensor("out", [C, NLOC], dt.float32, kind="ExternalOutput")

    with tile.TileContext(nc) as tc:
        from contextlib import ExitStack
        with ExitStack() as top:
            const = top.enter_context(tc.tile_pool(name="const", bufs=1))
            dram = top.enter_context(tc.tile_pool(name="dram", bufs=1, space="DRAM"))
            spool = top.enter_context(tc.tile_pool(name="small", bufs=1))
            res = top.enter_context(tc.tile_pool(name="res", bufs=1))
            inp = top.enter_context(tc.tile_pool(name="inp", bufs=4))

            # ---- constants to SBUF
            def load_c(d, shape, dty, cast=False, tag=None):
                t = const.tile(shape, dty, tag=tag or d.name + "_sb")
                (nc.gpsimd if cast else nc.sync).dma_start(t[:], d[:])
                return t

            wfeatT = load_c(wfeatT_d, [C, C1], dt.float16, cast=True)
            wgrpT = load_c(wgrpT_d, [C, C1], dt.float16, cast=True)
            wfoT = load_c(wfoT_d, [C, C], dt.float16, cast=True)
            w1T = load_c(w1T_d, [C, C], dt.float32)
            w2T = load_c(w2T_d, [C, C], dt.float32)
            bcat = load_c(bcat_d, [C, 1], dt.float32)
            b1 = load_c(b1_d, [C, 1], dt.float32)
            b2 = load_c(b2_d, [C, 1], dt.float32)
            bfo = load_c(bfo_d, [C, 1], dt.float32)
            gn = {nm: load_c(d, [C, 1], dt.float32) for nm, d in gn_d.items()}
            gi = load_c(gi_d, [C, G], dt.float32)
            git = load_c(git_d, [G, C], dt.float32)
            onesc = load_c(onesc_d, [1, C], dt.float16)
            featsb = const.tile([C, NLOC], dt.float16, tag="featsb")
            nc.gpsimd.dma_start(featsb[:], feat_d[:])

            st1 = spool.tile([C, NCH, 4, 6], dt.float32, tag="st1")
            st2 = spool.tile([C, 2 * NCH, 2, 6], dt.float32, tag="st2")
            st3 = spool.tile([C, 2 * NCH, 2, 6], dt.float32, tag="st3")
            outbuf = spool.tile([C, NLOC], dt.float32, tag="outbuf")
            vspill = dram.tile([C, FLOC], dt.float16, tag="vspill")

            u_tiles = []
            z3_tiles = []

            # ================= PHASE A =================
            with tc.tile_pool(name="psA", bufs=2, space="PSUM") as psA:
                for i in range(NCH):
                    gf_t = inp.tile([C, NPC, K], dt.float16, tag="instream")
                    nc.gpsimd.dma_start(gf_t[:], gf_d[:, i * NPC:(i + 1) * NPC, :])
                    ups = psA.tile([C, CHUNK], dt.float32, tag="ups")
                    for j in range(4):
                        rf = featsb[:, i * NPC + j * 16: i * NPC + (j + 1) * 16]
                        rf = rf.rearrange("c (n o) -> c n o", o=1).to_broadcast((C, 16, K))
                        nc.tensor.matmul(ups[0:C1, j * 512:(j + 1) * 512], wfeatT[:], rf,
                                         start=True, stop=True)
                        rg = gf_t[:, j * 16:(j + 1) * 16, :].rearrange("c n k -> c (n k)")
                        nc.tensor.matmul(ups[C1:C, j * 512:(j + 1) * 512], wgrpT[:], rg,
                                         start=True, stop=True)
                    u_t = res.tile([C, CHUNK], dt.float16, tag="resident", bufs=NCH)
                    nc.scalar.activation(u_t[:], ups[:], AF.Relu, bias=bcat[:])
                    for a in range(4):
                        nc.vector.bn_stats(st1[:, i, a, :],
                                           u_t[:, a * 512:(a + 1) * 512])
                    u_tiles.append(u_t)

            # ---- stats round 1 (GN1)
            with (tc.tile_pool(name="stats1", bufs=1) as sb1,
                  tc.tile_pool(name="statps1", bufs=1, space="PSUM") as ps1):
                agg = sb1.tile([C, 2], dt.float32, tag="agg")
                nc.vector.bn_aggr(agg[:], st1[:].rearrange("c a b s -> c (a b) s"))
                loc = sb1.tile([C, 2], dt.float32, tag="loc")
                nc.vector.tensor_scalar(loc[:, 0:1], agg[:, 0:1], float(FLOC), None, ALU.mult)
                tmp = sb1.tile([C, 1], dt.float32, tag="tmp")
                nc.vector.tensor_tensor(tmp[:], agg[:, 0:1], agg[:, 0:1], ALU.mult)
                nc.vector.tensor_tensor(tmp[:], agg[:, 1:2], tmp[:], ALU.add)
                nc.vector.tensor_scalar(loc[:, 1:2], tmp[:], float(FLOC), None, ALU.mult)
                bout = _allreduce(nc, dram, loc, 2, groups)
                tot = sb1.tile([C, 2], dt.float32, tag="tot")
                nc.sync.dma_start(tot[:], bout[:])
                (r1_pair,) = _stats_round(nc, tc, (sb1, ps1), tot, 2, gi, git)
                inv1, mu1 = r1_pair
                # fold GN1 into W1: W1effT[c,o] = W1T[c,o] * gn1w[c]*inv1[c]
                r1 = sb1.tile([C, 1], dt.float32, tag="r1")
                nc.vector.tensor_tensor(r1[:], gn["gn1w"][:], inv1[:], ALU.mult)
                t1 = sb1.tile([C, 1], dt.float32, tag="t1")
                nc.vector.tensor_tensor(t1[:], mu1[:], r1[:], ALU.mult)
                nc.vector.tensor_tensor(t1[:], gn["gn1b"][:], t1[:], ALU.subtract)
                w1e32 = sb1.tile([C, C], dt.float32, tag="w1e32")
                nc.vector.tensor_scalar(w1e32[:], w1T[:], r1[:], None, ALU.mult)
                w1e = const.tile([C, C], dt.float16, tag="w1e")
                nc.vector.tensor_copy(w1e[:], w1e32[:])
                bp = ps1.tile([C, 1], dt.float32, tag="bp")
                nc.tensor.matmul(bp[:], w1T[:], t1[:], start=True, stop=True)
                b1eff = const.tile([C, 1], dt.float32, tag="b1eff")
                nc.vector.tensor_tensor(b1eff[:], bp[:], b1[:], ALU.add)

            # ================= PHASE B =================
            with (tc.tile_pool(name="psB1", bufs=2, space="PSUM") as psB1,
                  tc.tile_pool(name="psB2", bufs=2, space="PSUM") as psB2,
                  tc.tile_pool(name="vout", bufs=3) as vout):
                for i in range(NCH):
                    # z2/v path from resident u
                    for h in range(2):
                        z2p = psB1.tile([C, 1024], dt.float32, tag="z2p")
                        for j in range(2):
                            nc.tensor.matmul(
                                z2p[:, j * 512:(j + 1) * 512], w1e[:],
                                u_tiles[i][:, h * 1024 + j * 512: h * 1024 + (j + 1) * 512],
                                start=True, stop=True)
                        v_t = vout.tile([C, 1024], dt.float16, tag="v_t")
                        nc.scalar.activation(v_t[:], z2p[:], AF.Relu, bias=b1eff[:])
                        for a in range(2):
                            nc.vector.bn_stats(st2[:, 2 * i + h, a, :],
                                               v_t[:, a * 512:(a + 1) * 512])
                        nc.sync.dma_start(
                            vspill[:, i * CHUNK + h * 1024: i * CHUNK + (h + 1) * 1024],
                            v_t[:])
                    # z3 path from streamed gfo
                    gfo_t = inp.tile([C, NPC, K], dt.float16, tag="instream")
                    nc.gpsimd.dma_start(gfo_t[:], gfo_d[:, i * NPC:(i + 1) * NPC, :])
                    z3_t = res.tile([C, CHUNK], dt.float16, tag="resident", bufs=NCH)
                    for h in range(2):
                        z3p = psB2.tile([C, 1024], dt.float32, tag="z3p")
                        for j in range(2):
                            rg = gfo_t[:, h * 32 + j * 16: h * 32 + (j + 1) * 16, :]
                            nc.tensor.matmul(z3p[:, j * 512:(j + 1) * 512], wfoT[:],
                                             rg.rearrange("c n k -> c (n k)"),
                                             start=True, stop=True)
                        nc.scalar.activation(z3_t[:, h * 1024:(h + 1) * 1024], z3p[:],
                                             AF.Identity, bias=bfo[:])
                        for a in range(2):
                            nc.vector.bn_stats(
                                st3[:, 2 * i + h, a, :],
                                z3_t[:, h * 1024 + a * 512: h * 1024 + (a + 1) * 512])
                    z3_tiles.append(z3_t)

            # ---- stats round 2 (GN2 + GN3)
            with (tc.tile_pool(name="stats2", bufs=1) as sb2,
                  tc.tile_pool(name="statps2", bufs=1, space="PSUM") as ps2):
                loc2 = sb2.tile([C, 4], dt.float32, tag="loc2")
                for cidx, st in ((0, st2), (2, st3)):
                    agg = sb2.tile([C, 2], dt.float32, tag="agg2")
                    nc.vector.bn_aggr(agg[:], st[:].rearrange("c a b s -> c (a b) s"))
                    nc.vector.tensor_scalar(loc2[:, cidx:cidx + 1], agg[:, 0:1],
                                            float(FLOC), None, ALU.mult)
                    tmp = sb2.tile([C, 1], dt.float32, tag="tmp2")
                    nc.vector.tensor_tensor(tmp[:], agg[:, 0:1], agg[:, 0:1], ALU.mult)
                    nc.vector.tensor_tensor(tmp[:], agg[:, 1:2], tmp[:], ALU.add)
                    nc.vector.tensor_scalar(loc2[:, cidx + 1:cidx + 2], tmp[:],
                                            float(FLOC), None, ALU.mult)
                bout2 = _allreduce(nc, dram, loc2, 4, groups)
                tot2 = sb2.tile([C, 4], dt.float32, tag="tot2")
                nc.sync.dma_start(tot2[:], bout2[:])
                pairs = _stats_round(nc, tc, (sb2, ps2), tot2, 4, gi, git)
                (inv2, mu2), (inv3, mu3) = pairs
                # fold GN2 into W2
                r2 = sb2.tile([C, 1], dt.float32, tag="r2")
                nc.vector.tensor_tensor(r2[:], gn["gn2w"][:], inv2[:], ALU.mult)
                t2 = sb2.tile([C, 1], dt.float32, tag="t2")
                nc.vector.tensor_tensor(t2[:], mu2[:], r2[:], ALU.mult)
                nc.vector.tensor_tensor(t2[:], gn["gn2b"][:], t2[:], ALU.subtract)
                w2e32 = sb2.tile([C, C], dt.float32, tag="w2e32")
                nc.vector.tensor_scalar(w2e32[:], w2T[:], r2[:], None, ALU.mult)
                w2e = const.tile([C, C], dt.float16, tag="w2e")
                nc.vector.tensor_copy(w2e[:], w2e32[:])
                bp2 = ps2.tile([C, 1], dt.float32, tag="bp2")
                nc.tensor.matmul(bp2[:], w2T[:], t2[:], start=True, stop=True)
                b2eff = const.tile([C, 1], dt.float32, tag="b2eff")
                nc.vector.tensor_tensor(b2eff[:], bp2[:], b2[:], ALU.add)
                # GN3 affine on z3
                s3 = const.tile([C, 1], dt.float32, tag="s3")
                nc.vector.tensor_tensor(s3[:], gn["gn3w"][:], inv3[:], ALU.mult)
                t3 = const.tile([C, 1], dt.float32, tag="t3")
                nc.vector.tensor_tensor(t3[:], mu3[:], s3[:], ALU.mult)
                nc.vector.tensor_tensor(t3[:], gn["gn3b"][:], t3[:], ALU.subtract)

            # ================= PHASE C =================
            with (tc.tile_pool(name="psC", bufs=2, space="PSUM") as psC,
                  tc.tile_pool(name="cpool", bufs=2) as cp):
                for i in range(NCH):
                    v_t = inp.tile([C, CHUNK], dt.float16, tag="instream")
                    nc.sync.dma_start(v_t[:], vspill[:, i * CHUNK:(i + 1) * CHUNK])
                    bn_t = cp.tile([1, CHUNK], dt.float16, tag="bn_t")
                    nc.sync.dma_start(bn_t[:], bigneg_d[i:i + 1, :])
                    scp = psC.tile([C, CHUNK], dt.float32, tag="scp")
                    for j in range(4):
                        nc.tensor.matmul(scp[:, j * 512:(j + 1) * 512], w2e[:],
                                         v_t[:, j * 512:(j + 1) * 512],
                                         start=True, stop=False)
                    for h in range(4):
                        nc.tensor.matmul(scp[:, h * 512:(h + 1) * 512], onesc[:],
                                         bn_t[:, h * 512:(h + 1) * 512],
                                         start=False, stop=True)
                    p_t = cp.tile([C, NPC, K], dt.float16, tag="p_t")
                    nc.scalar.activation(p_t[:].rearrange("c n k -> c (n k)"), scp[:],
                                         AF.Exp, bias=b2eff[:])
                    ga_t = cp.tile([C, NPC, K], dt.float16, tag="ga_t")
                    nc.scalar.activation(ga_t[:].rearrange("c n k -> c (n k)"),
                                         z3_tiles[i][:], AF.Relu,
                                         bias=t3[:], scale=s3[:])
                    m_t = cp.tile([C, NPC, K], dt.float16, tag="m_t", bufs=1)
                    nc.vector.tensor_tensor(m_t[:], p_t[:], ga_t[:], ALU.mult)
                    # tree reduces over k
                    acc = {}
                    for nm, src in (("den", p_t), ("num", m_t)):
                        cur = src
                        w = K
                        while w > 2:
                            nxt = cp.tile([C, NPC, w // 2], dt.float16,
                                          tag=f"{nm}{w // 2}", bufs=1)
                            nc.vector.tensor_tensor(nxt[:], cur[:, :, 0:w // 2],
                                                    cur[:, :, w // 2:w], ALU.add)
                            cur = nxt
                            w //= 2
                        fin = cp.tile([C, NPC], dt.float32, tag=f"{nm}f", bufs=1)
                        nc.vector.tensor_tensor(
                            fin[:].rearrange("c (n o) -> c n o", o=1),
                            cur[:, :, 0:1], cur[:, :, 1:2], ALU.add)
                        acc[nm] = fin
                    rec = cp.tile([C, NPC], dt.float32, tag="rec")
                    nc.vector.reciprocal(rec[:], acc["den"][:])
                    nc.vector.tensor_tensor(outbuf[:, i * NPC:(i + 1) * NPC],
                                            acc["num"][:], rec[:], ALU.mult)
            nc.sync.dma_start(out_d[:], outbuf[:])

    nc.compile()
    _CACHE[n_cores] = nc
    return nc


def _host_prep(inputs, n_cores=8):
    """Slice full inputs into per-core input maps."""
    feat = np.ascontiguousarray(np.asarray(inputs['feat'], dtype=np.float32))
    gf = np.asarray(inputs['grouped_feat'], dtype=np.float32)
    gfo = np.asarray(inputs['grouped_feat_out'], dtype=np.float32)
    count = np.asarray(inputs['count'])
    cnt = np.clip(count, 1, None)
    mask_neg = np.where(np.arange(K)[None, None, :] < cnt[:, :, None],
                        np.float16(0), np.float16(MASKNEG))  # (B, N, K) f16

    wfeatT = np.ascontiguousarray(np.asarray(inputs['W_feat'], np.float32).T)
    wgrpT = np.ascontiguousarray(np.asarray(inputs['W_grp'], np.float32).T)
    w1T = np.ascontiguousarray(np.asarray(inputs['W_wc1'], np.float32).T)
    w2T = np.ascontiguousarray(np.asarray(inputs['W_wc2'], np.float32).T)
    wfoT = np.ascontiguousarray(np.asarray(inputs['W_fo'], np.float32).T)
    bcat = np.concatenate([np.asarray(inputs['b_feat'], np.float32),
                           np.asarray(inputs['b_grp'], np.float32)]).reshape(C, 1)
    b1 = np.asarray(inputs['b_wc1'], np.float32).reshape(C, 1)
    b2 = np.asarray(inputs['b_wc2'], np.float32).reshape(C, 1)
    bfo = np.asarray(inputs['b_fo'], np.float32).reshape(C, 1)
    gn = {"gn1w": inputs['gn1_w'], "gn1b": inputs['gn1_b'],
          "gn2w": inputs['gn2_w'], "gn2b": inputs['gn2_b'],
          "gn3w": inputs['gn3_w'], "gn3b": inputs['gn3_b']}
    gn = {k: np.asarray(v, np.float32).reshape(C, 1) for k, v in gn.items()}
    gi = np.zeros((C, G), np.float32)
    gi[np.arange(C), np.arange(C) // CPG] = 1.0
    git = np.ascontiguousarray(gi.T)
    onesc = np.ones((1, C), np.float16)

    shared = dict(wfeatT=wfeatT, wgrpT=wgrpT, w1T=w1T, w2T=w2T, wfoT=wfoT,
                  bcat=bcat, b1=b1, b2=b2, bfo=bfo, gi=gi, git=git,
                  onesc=onesc, **gn)
    in_maps = []
    for core in range(n_cores):
        b = core // 2
        half = core % 2
        lo, hi = half * NLOC, (half + 1) * NLOC
        m = dict(shared)
        m['gf'] = np.ascontiguousarray(gf[b, :, lo:hi, :])
        m['gfo'] = np.ascontiguousarray(gfo[b, :, lo:hi, :])
        m['feat'] = np.ascontiguousarray(feat[b, :, lo:hi])
        m['bigneg'] = np.ascontiguousarray(
            mask_neg[b, lo:hi, :].reshape(NCH, CHUNK))
        in_maps.append(m)
    return in_maps


def _gather(results, n_cores=8):
    out = np.zeros((B, C, N), np.float32)
    for core in range(n_cores):
        b = core // 2
        half = core % 2
        out[b, :, half * NLOC:(half + 1) * NLOC] = results[core]["out"]
    return out


def run(inputs, trace=False):
    n_cores = 8
    nc = _build(n_cores)
    in_maps = _host_prep(inputs, n_cores)
    res = run_bass_kernel_spmd(nc, in_maps, list(range(n_cores)), trace=trace)
    return _gather(res.results, n_cores), res


def kernel(**inputs) -> np.ndarray:
    out, _ = run(inputs, trace=False)
    return out
